# revision 1
# baseline (speedup 1.0000x reference)
"""MoE FFN (top-2 of 8 experts) on 8 Trainium2 NeuronCores.

Strategy (expert parallelism, per the sharding hint):
  - Host: router (softmax -> top-2 -> renorm) on [T, 8] logits — negligible
    FLOPs — then dispatch: gather each expert's tokens, transpose to [D, C]
    so the device needs no on-chip transposes at all.
  - Device (SPMD, one expert per core): hT = gelu(w1.T-accumulated matmul)
    with F on the partition axis (b1 becomes a per-partition activation
    bias), then y = hT.T @ w2 with hT used directly as the stationary
    operand, scaled by the per-token combine weight on the way out of PSUM.
    All matmuls bf16 with f32 PSUM accumulation.
  - Host: scatter-add the two expert contributions per token, plus the
    analytic sum_e cw[e,t]*b2[e] term.
"""

import os
import sys

sys.path.insert(0, "/opt/trn_rl_repo")

import numpy as np
import ml_dtypes

import concourse.bass as bass
import concourse.bacc as bacc
import concourse.mybir as mybir
from concourse import tile
from concourse.bass_utils import run_bass_kernel_spmd

BF16 = ml_dtypes.bfloat16
P = 128
D, F, E = 1024, 4096, 8
ND, NF = D // P, F // P  # 8, 32
TOP_K = 2

TRACE = bool(int(os.environ.get("MOE_TRACE", "0")))
TRACE_ALL = bool(int(os.environ.get("MOE_TRACE_ALL", "0")))
LAST = {}

_BUILD_CACHE = {}


def _enable_axon_profiling():
    """The image's antenv lacks axon_hooks, so boot() silently skipped NTFF
    hook registration. Recreate the module and register the ctypes hook so
    run_bass_kernel_spmd(trace=True) can profile. Also keep artifacts local."""
    import types
    import contextlib

    if "antenv.axon_hooks" not in sys.modules:
        mod = types.ModuleType("antenv.axon_hooks")
        mod._hook = None

        def set_axon_ntff_profile_hook(h):
            mod._hook = h

        def get_axon_ntff_profile_hook():
            return mod._hook

        mod.set_axon_ntff_profile_hook = set_axon_ntff_profile_hook
        mod.get_axon_ntff_profile_hook = get_axon_ntff_profile_hook
        sys.modules["antenv.axon_hooks"] = mod
        import antenv

        antenv.axon_hooks = mod
    hooks = sys.modules["antenv.axon_hooks"]
    if hooks.get_axon_ntff_profile_hook() is None:
        from trn_agent_boot.trn_boot import _ntff_profile_via_ctypes

        hooks.set_axon_ntff_profile_hook(
            _ntff_profile_via_ctypes("/opt/axon/libaxon_pjrt.so")
        )
    import concourse.bass_utils as bu

    bu.upload_artifacts = lambda tmpdir: tmpdir


if TRACE:
    _enable_axon_profiling()


CC = 512


def _chunks_for(C):
    ch = [CC] * (C // CC)
    if C % CC:
        ch.append(C % CC)
    return ch


def _build(C, act_func=None):
    """One expert's FFN over C (padded) tokens; SPMD across 8 cores."""
    if act_func is None:
        act_func = mybir.ActivationFunctionType.Gelu
    nc = bacc.Bacc()
    dt = mybir.dt
    xTc = nc.dram_tensor("xTc", [P, ND, C], dt.bfloat16, kind="ExternalInput")
    w1c = nc.dram_tensor("w1c", [P, NF // 4, ND, 512], dt.bfloat16, kind="ExternalInput")
    w2c = nc.dram_tensor("w2c", [P, NF, D], dt.bfloat16, kind="ExternalInput")
    b1c = nc.dram_tensor("b1c", [P, NF], dt.float32, kind="ExternalInput")
    cwc = nc.dram_tensor("cwc", [P, C // P], dt.float32, kind="ExternalInput")
    y = nc.dram_tensor("y", [C, D], dt.float32, kind="ExternalOutput")

    chunks = _chunks_for(C)
    with tile.TileContext(nc) as tc:
        with (
            tc.tile_pool(name="weights", bufs=1) as wpool,
            tc.tile_pool(name="consts", bufs=1) as cpool,
            tc.tile_pool(name="xin", bufs=2) as xpool,
            tc.tile_pool(name="hmid", bufs=1) as hpool,
            tc.tile_pool(name="yout", bufs=3) as ypool,
            tc.tile_pool(name="psh", bufs=4, space="PSUM") as psh,
            tc.tile_pool(name="psy", bufs=4, space="PSUM") as psy,
        ):
            # Weights as separate tiles per slice: dependency tracking is
            # tile-granular, so per-kd/per-group tiles let each matmul wait
            # only on its own slice's DMA instead of the full weight load.
            w1_sb = [wpool.tile([P, ND, 512], dt.bfloat16, name=f"w1_{g}", tag=f"w1_{g}") for g in range(NF // 4)]
            w2_sb = [wpool.tile([P, 4, D], dt.bfloat16, name=f"w2_{g}", tag=f"w2_{g}") for g in range(NF // 4)]
            b1_sb = cpool.tile([P, NF], dt.float32)
            cw_sb = cpool.tile([P, C // P], dt.float32)

            # chunk-0 activations first (small, on the critical path to the
            # very first matmul), then w1; w2 is issued just-in-time inside
            # chunk 0's matmul-1 loop so it doesn't steal load bandwidth.
            # per-kd tiles for chunk 0 so the first matmul only waits on the
            # first 128KB slice, not the whole chunk.
            xT0_sb = xpool.tile([P, ND, CC], dt.bfloat16, tag="xT")
            nc.sync.dma_start(
                out=xT0_sb[:, :, : chunks[0]], in_=xTc[:, :, : chunks[0]]
            )
            warm_l = cpool.tile([P, P], dt.bfloat16)
            nc.vector.memset(warm_l[:], 0.0)
            warm_ps = psy.tile([P, 512], dt.float32, tag="py")
            for i in range(20):
                nc.tensor.matmul(
                    warm_ps[:, :P], warm_l[:], warm_l[:],
                    start=(i == 0), stop=(i == 19),
                )

            for g in range(NF // 4):
                nc.sync.dma_start(out=w1_sb[g][:], in_=w1c[:, g, :, :])
            nc.sync.dma_start(out=b1_sb[:], in_=b1c[:])
            nc.sync.dma_start(out=cw_sb[:], in_=cwc[:])

            c0 = 0
            for ci, Cc in enumerate(chunks):
                ncb = Cc // P
                if ci == 0:
                    xT_sb = xT0_sb
                else:
                    xT_sb = xpool.tile([P, ND, CC], dt.bfloat16, tag="xT")
                    nc.sync.dma_start(
                        out=xT_sb[:, :, :Cc], in_=xTc[:, :, c0 : c0 + Cc]
                    )
                hT_sb = hpool.tile([P, NF, CC], dt.bfloat16, tag="hT")
                for fb in range(NF):
                    if ci == 0 and fb == 8:
                        for g in range(NF // 4):
                            nc.sync.dma_start(
                                out=w2_sb[g][:],
                                in_=w2c[:, g * 4 : (g + 1) * 4, :],
                            )
                    ph = psh.tile([P, CC], dt.float32, tag="ph")
                    for kd in range(ND):
                        nc.tensor.matmul(
                            ph[:, :Cc],
                            w1_sb[fb // 4][:, kd, (fb % 4) * P : (fb % 4 + 1) * P],
                            xT_sb[:, kd, :Cc],
                            start=(kd == 0),
                            stop=(kd == ND - 1),
                        )
                    nc.scalar.activation(
                        hT_sb[:, fb, :Cc],
                        ph[:, :Cc],
                        act_func,
                        bias=b1_sb[:, fb : fb + 1],
                    )
                for cb in range(ncb):
                    y_sb = ypool.tile([P, D], dt.float32, tag="y")
                    for dc in range(2):
                        py = psy.tile([P, 512], dt.float32, tag="py")
                        for fb in range(NF):
                            nc.tensor.matmul(
                                py[:],
                                hT_sb[:, fb, cb * P : (cb + 1) * P],
                                w2_sb[fb // 4][:, fb % 4, dc * 512 : (dc + 1) * 512],
                                start=(fb == 0),
                                stop=(fb == NF - 1),
                            )
                        blk = c0 // P + cb
                        nc.vector.tensor_scalar_mul(
                            y_sb[:, dc * 512 : (dc + 1) * 512],
                            py[:],
                            cw_sb[:, blk : blk + 1],
                        )
                        nc.sync.dma_start(
                            out=y[
                                c0 + cb * P : c0 + (cb + 1) * P,
                                dc * 512 : (dc + 1) * 512,
                            ],
                            in_=y_sb[:, dc * 512 : (dc + 1) * 512],
                        )
                c0 += Cc
    nc.compile()
    return nc


def _route(xf, router_w, router_b):
    """Replicates reference routing in numpy f32."""
    logits = xf @ router_w + router_b
    logits = logits - logits.max(axis=1, keepdims=True)
    p = np.exp(logits)
    p /= p.sum(axis=1, keepdims=True)
    top_i = np.argsort(-p, axis=1, kind="stable")[:, :TOP_K]
    tp = np.take_along_axis(p, top_i, 1)
    tp = tp / tp.sum(axis=1, keepdims=True)
    return top_i, tp.astype(np.float32)


def kernel(x, w1, b1, w2, b2, router_w, router_b):
    x = np.asarray(x, np.float32)
    B, S, _ = x.shape
    T = B * S
    xf = x.reshape(T, D)
    w1f = np.asarray(w1, np.float32)
    w2f = np.asarray(w2, np.float32)
    b1f = np.asarray(b1, np.float32)
    b2f = np.asarray(b2, np.float32)

    top_i, tp = _route(xf, np.asarray(router_w, np.float32), np.asarray(router_b, np.float32))

    idxs, cws = [], []
    for e in range(E):
        sel = top_i == e
        rows = np.nonzero(sel.any(axis=1))[0]
        w = (tp * sel).sum(axis=1)[rows]
        idxs.append(rows)
        cws.append(w.astype(np.float32))

    maxn = max(len(r) for r in idxs)
    C = max(CC, ((maxn + 127) // 128) * 128)

    if C not in _BUILD_CACHE:
        _BUILD_CACHE[C] = _build(C)
    nc = _BUILD_CACHE[C]

    w1b = w1f.astype(BF16)
    w2b = w2f.astype(BF16)
    in_maps = []
    for e in range(E):
        n = len(idxs[e])
        xT = np.zeros((P, ND, C), BF16)
        if n:
            g = xf[idxs[e]].astype(BF16).T  # [D, n]
            xT[:, :, :n] = g.reshape(ND, P, n).transpose(1, 0, 2)
        cwf = np.zeros(C, np.float32)
        cwf[:n] = cws[e]
        in_maps.append(
            {
                "xTc": xT,
                "w1c": np.ascontiguousarray(w1b[e].reshape(ND, P, NF // 4, 512).transpose(1, 2, 0, 3)),
                "w2c": np.ascontiguousarray(w2b[e].reshape(NF, P, D).transpose(1, 0, 2)),
                "b1c": np.ascontiguousarray(b1f[e].reshape(NF, P).T),
                "cwc": np.ascontiguousarray(cwf.reshape(C // P, P).T),
            }
        )

    res = run_bass_kernel_spmd(
        nc,
        in_maps,
        list(range(E)),
        trace=TRACE,
        trace_cores=list(range(E)) if TRACE_ALL else None,
    )
    LAST["exec_time_ns"] = res.exec_time_ns
    LAST["res"] = res
    LAST["C"] = C

    outf = np.zeros((T, D), np.float32)
    for e in range(E):
        n = len(idxs[e])
        if n:
            ye = np.asarray(res.results[e]["y"], np.float32)
            outf[idxs[e]] += ye[:n]
    # b2 enters as sum_e cw[e,t] * b2[e]
    cw_dense = np.zeros((T, E), np.float32)
    np.put_along_axis(cw_dense, top_i, tp, axis=1)
    outf += cw_dense @ b2f
    return outf.reshape(B, S, D)



# revision 2
# speedup vs baseline: 1.1773x; 1.1773x over previous
"""MoE FFN (top-2 of 8 experts) on 8 Trainium2 NeuronCores.

Strategy (expert parallelism, per the sharding hint):
  - Host: router (softmax -> top-2 -> renorm) on [T, 8] logits — negligible
    FLOPs — then dispatch: gather each expert's tokens, transpose to [D, C]
    so the device needs no on-chip transposes at all.
  - Device (SPMD, one expert per core): hT = gelu(w1.T-accumulated matmul)
    with F on the partition axis (b1 becomes a per-partition activation
    bias), then y = hT.T @ w2 with hT used directly as the stationary
    operand, scaled by the per-token combine weight on the way out of PSUM.
    All matmuls bf16 with f32 PSUM accumulation.
  - Host: scatter-add the two expert contributions per token, plus the
    analytic sum_e cw[e,t]*b2[e] term.

DMA orchestration: w1 is staged fb-major (32 tiles of [P, ND, 128]) so the
first matmul group only waits on a 256KB transfer and delivery stays ahead
of consumption; x chunk 0 is split per-kd for the same reason. w2 streams
in during chunk 0's first matmul phase.
"""

import os
import sys

sys.path.insert(0, "/opt/trn_rl_repo")

import numpy as np
import ml_dtypes

import concourse.bass as bass
import concourse.bacc as bacc
import concourse.mybir as mybir
from concourse import tile
from concourse.bass_utils import run_bass_kernel_spmd

BF16 = ml_dtypes.bfloat16
P = 128
D, F, E = 1024, 4096, 8
ND, NF = D // P, F // P  # 8, 32
TOP_K = 2

TRACE = bool(int(os.environ.get("MOE_TRACE", "0")))
TRACE_ALL = bool(int(os.environ.get("MOE_TRACE_ALL", "0")))
LAST = {}

_BUILD_CACHE = {}


def _enable_axon_profiling():
    """The image's antenv lacks axon_hooks, so boot() silently skipped NTFF
    hook registration. Recreate the module and register the ctypes hook so
    run_bass_kernel_spmd(trace=True) can profile. Also keep artifacts local."""
    import types

    if "antenv.axon_hooks" not in sys.modules:
        mod = types.ModuleType("antenv.axon_hooks")
        mod._hook = None

        def set_axon_ntff_profile_hook(h):
            mod._hook = h

        def get_axon_ntff_profile_hook():
            return mod._hook

        mod.set_axon_ntff_profile_hook = set_axon_ntff_profile_hook
        mod.get_axon_ntff_profile_hook = get_axon_ntff_profile_hook
        sys.modules["antenv.axon_hooks"] = mod
        import antenv

        antenv.axon_hooks = mod
    hooks = sys.modules["antenv.axon_hooks"]
    if hooks.get_axon_ntff_profile_hook() is None:
        from trn_agent_boot.trn_boot import _ntff_profile_via_ctypes

        hooks.set_axon_ntff_profile_hook(
            _ntff_profile_via_ctypes("/opt/axon/libaxon_pjrt.so")
        )
    import concourse.bass_utils as bu

    bu.upload_artifacts = lambda tmpdir: tmpdir


if TRACE:
    _enable_axon_profiling()


CC = 512


def _chunks_for(C):
    ch = [CC] * (C // CC)
    if C % CC:
        ch.append(C % CC)
    return ch


def _build(C, act_func=None):
    """One expert's FFN over C (padded) tokens; SPMD across 8 cores."""
    if act_func is None:
        act_func = mybir.ActivationFunctionType.Gelu
    nc = bacc.Bacc()
    dt = mybir.dt
    xTc = nc.dram_tensor("xTc", [P, ND, C], dt.bfloat16, kind="ExternalInput")
    w1c = nc.dram_tensor("w1c", [P, NF, ND, P], dt.bfloat16, kind="ExternalInput")
    w2c = nc.dram_tensor("w2c", [P, NF, D], dt.bfloat16, kind="ExternalInput")
    b1c = nc.dram_tensor("b1c", [P, NF], dt.float32, kind="ExternalInput")
    cwc = nc.dram_tensor("cwc", [P, C // P], dt.float32, kind="ExternalInput")
    y = nc.dram_tensor("y", [C, D], dt.float32, kind="ExternalOutput")

    chunks = _chunks_for(C)
    with tile.TileContext(nc) as tc:
        with (
            tc.tile_pool(name="weights", bufs=1) as wpool,
            tc.tile_pool(name="consts", bufs=1) as cpool,
            tc.tile_pool(name="xin", bufs=2) as xpool,
            tc.tile_pool(name="hmid", bufs=1) as hpool,
            tc.tile_pool(name="yout", bufs=3) as ypool,
            tc.tile_pool(name="psh", bufs=4, space="PSUM") as psh,
            tc.tile_pool(name="psy", bufs=4, space="PSUM") as psy,
        ):
            # w1 fb-major: one small tile per F-block so the first matmul
            # group waits only on 256KB, and DMA delivery (0.7us/tile)
            # outruns PE consumption (1.7us/tile).
            w1_sb = [wpool.tile([P, ND, P], dt.bfloat16, name=f"w1_{fb}", tag=f"w1_{fb}") for fb in range(NF)]
            w2_sb = [wpool.tile([P, 4, D], dt.bfloat16, name=f"w2_{g}", tag=f"w2_{g}") for g in range(NF // 4)]
            b1_sb = cpool.tile([P, NF], dt.float32)
            cw_sb = cpool.tile([P, C // P], dt.float32)

            # PE warmup (p-state ramp) on memset data, overlapping the DMAs.
            warm_l = cpool.tile([P, P], dt.bfloat16)
            nc.vector.memset(warm_l[:], 0.0)
            warm_ps = psy.tile([P, 512], dt.float32, tag="py")
            for i in range(10):
                nc.tensor.matmul(
                    warm_ps[:, :P], warm_l[:], warm_l[:],
                    start=(i == 0), stop=(i == 9),
                )

            # DMA issue order = consumption order: w1 fb0, then x chunk 0
            # per-kd (first matmul needs only kd0), then the rest of w1.
            xT0_sb = xpool.tile([P, ND, CC], dt.bfloat16, tag="xT")
            nc.sync.dma_start(out=w1_sb[0][:], in_=w1c[:, 0])
            for kd in range(ND):
                nc.sync.dma_start(
                    out=xT0_sb[:, kd, : chunks[0]], in_=xTc[:, kd, : chunks[0]]
                )
            for fb in range(1, NF):
                nc.sync.dma_start(out=w1_sb[fb][:], in_=w1c[:, fb])
            nc.sync.dma_start(out=b1_sb[:], in_=b1c[:])
            nc.sync.dma_start(out=cw_sb[:], in_=cwc[:])

            c0 = 0
            for ci, Cc in enumerate(chunks):
                ncb = Cc // P
                if ci == 0:
                    xT_sb = xT0_sb
                else:
                    xT_sb = xpool.tile([P, ND, CC], dt.bfloat16, tag="xT")
                    nc.sync.dma_start(
                        out=xT_sb[:, :, :Cc], in_=xTc[:, :, c0 : c0 + Cc]
                    )
                hT_sb = hpool.tile([P, NF, CC], dt.bfloat16, tag="hT")
                for fb in range(NF):
                    if ci == 0 and fb == 6:
                        # w2 streams in while chunk 0's m1 still has ~45us
                        # of work; ready well before m2 starts.
                        for g in range(NF // 4):
                            nc.sync.dma_start(
                                out=w2_sb[g][:],
                                in_=w2c[:, g * 4 : (g + 1) * 4, :],
                            )
                    ph = psh.tile([P, CC], dt.float32, tag="ph")
                    for kd in range(ND):
                        nc.tensor.matmul(
                            ph[:, :Cc],
                            w1_sb[fb][:, kd, :],
                            xT_sb[:, kd, :Cc],
                            start=(kd == 0),
                            stop=(kd == ND - 1),
                        )
                    nc.scalar.activation(
                        hT_sb[:, fb, :Cc],
                        ph[:, :Cc],
                        act_func,
                        bias=b1_sb[:, fb : fb + 1],
                    )
                for cb in range(ncb):
                    y_sb = ypool.tile([P, D], dt.float32, tag="y")
                    for dc in range(2):
                        py = psy.tile([P, 512], dt.float32, tag="py")
                        for fb in range(NF):
                            nc.tensor.matmul(
                                py[:],
                                hT_sb[:, fb, cb * P : (cb + 1) * P],
                                w2_sb[fb // 4][:, fb % 4, dc * 512 : (dc + 1) * 512],
                                start=(fb == 0),
                                stop=(fb == NF - 1),
                            )
                        blk = c0 // P + cb
                        nc.vector.tensor_scalar_mul(
                            y_sb[:, dc * 512 : (dc + 1) * 512],
                            py[:],
                            cw_sb[:, blk : blk + 1],
                        )
                        nc.sync.dma_start(
                            out=y[
                                c0 + cb * P : c0 + (cb + 1) * P,
                                dc * 512 : (dc + 1) * 512,
                            ],
                            in_=y_sb[:, dc * 512 : (dc + 1) * 512],
                        )
                c0 += Cc
    nc.compile()
    return nc


def _route(xf, router_w, router_b):
    """Replicates reference routing in numpy f32."""
    logits = xf @ router_w + router_b
    logits = logits - logits.max(axis=1, keepdims=True)
    p = np.exp(logits)
    p /= p.sum(axis=1, keepdims=True)
    top_i = np.argsort(-p, axis=1, kind="stable")[:, :TOP_K]
    tp = np.take_along_axis(p, top_i, 1)
    tp = tp / tp.sum(axis=1, keepdims=True)
    return top_i, tp.astype(np.float32)


def kernel(x, w1, b1, w2, b2, router_w, router_b):
    x = np.asarray(x, np.float32)
    B, S, _ = x.shape
    T = B * S
    xf = x.reshape(T, D)
    w1f = np.asarray(w1, np.float32)
    w2f = np.asarray(w2, np.float32)
    b1f = np.asarray(b1, np.float32)
    b2f = np.asarray(b2, np.float32)

    top_i, tp = _route(xf, np.asarray(router_w, np.float32), np.asarray(router_b, np.float32))

    idxs, cws = [], []
    for e in range(E):
        sel = top_i == e
        rows = np.nonzero(sel.any(axis=1))[0]
        w = (tp * sel).sum(axis=1)[rows]
        idxs.append(rows)
        cws.append(w.astype(np.float32))

    maxn = max(len(r) for r in idxs)
    C = max(CC, ((maxn + 127) // 128) * 128)

    if C not in _BUILD_CACHE:
        _BUILD_CACHE[C] = _build(C)
    nc = _BUILD_CACHE[C]

    w1b = w1f.astype(BF16)
    w2b = w2f.astype(BF16)
    in_maps = []
    for e in range(E):
        n = len(idxs[e])
        xT = np.zeros((P, ND, C), BF16)
        if n:
            g = xf[idxs[e]].astype(BF16).T  # [D, n]
            xT[:, :, :n] = g.reshape(ND, P, n).transpose(1, 0, 2)
        cwf = np.zeros(C, np.float32)
        cwf[:n] = cws[e]
        in_maps.append(
            {
                "xTc": xT,
                # [P, NF, ND, P]: w1c[p, fb, kd, c] = w1[kd*P + p, fb*P + c]
                "w1c": np.ascontiguousarray(w1b[e].reshape(ND, P, NF, P).transpose(1, 2, 0, 3)),
                "w2c": np.ascontiguousarray(w2b[e].reshape(NF, P, D).transpose(1, 0, 2)),
                "b1c": np.ascontiguousarray(b1f[e].reshape(NF, P).T),
                "cwc": np.ascontiguousarray(cwf.reshape(C // P, P).T),
            }
        )

    res = run_bass_kernel_spmd(
        nc,
        in_maps,
        list(range(E)),
        trace=TRACE,
        trace_cores=list(range(E)) if TRACE_ALL else None,
    )
    LAST["exec_time_ns"] = res.exec_time_ns
    LAST["res"] = res
    LAST["C"] = C

    outf = np.zeros((T, D), np.float32)
    for e in range(E):
        n = len(idxs[e])
        if n:
            ye = np.asarray(res.results[e]["y"], np.float32)
            outf[idxs[e]] += ye[:n]
    # b2 enters as sum_e cw[e,t] * b2[e]
    cw_dense = np.zeros((T, E), np.float32)
    np.put_along_axis(cw_dense, top_i, tp, axis=1)
    outf += cw_dense @ b2f
    return outf.reshape(B, S, D)


# revision 5
# speedup vs baseline: 1.1912x; 1.0118x over previous
"""MoE FFN (top-2 of 8 experts) on 8 Trainium2 NeuronCores.

Strategy (expert parallelism, per the sharding hint):
  - Host: router (softmax -> top-2 -> renorm) on [T, 8] logits — negligible
    FLOPs — then dispatch: gather each expert's tokens, transpose to [D, C]
    so the device needs no on-chip transposes at all.
  - Device (SPMD, one expert per core): hT = gelu(w1.T-accumulated matmul)
    with F on the partition axis (b1 becomes a per-partition activation
    bias), then y = hT.T @ w2 with hT used directly as the stationary
    operand, scaled by the per-token combine weight on the way out of PSUM.
    All matmuls bf16 with f32 PSUM accumulation.
  - Host: scatter-add the two expert contributions per token, plus the
    analytic sum_e cw[e,t]*b2[e] term.

DMA orchestration: w1 is staged fb-major (32 tiles of [P, ND, 128]) so the
first matmul group only waits on a 256KB transfer and delivery stays ahead
of consumption; x chunk 0 is split per-kd for the same reason. w2 streams
in during chunk 0's first matmul phase.
"""

import os
import sys

sys.path.insert(0, "/opt/trn_rl_repo")

import numpy as np
import ml_dtypes

import concourse.bass as bass
import concourse.bacc as bacc
import concourse.mybir as mybir
from concourse import tile
from concourse.bass_utils import run_bass_kernel_spmd

BF16 = ml_dtypes.bfloat16
P = 128
D, F, E = 1024, 4096, 8
ND, NF = D // P, F // P  # 8, 32
TOP_K = 2

TRACE = bool(int(os.environ.get("MOE_TRACE", "0")))
TRACE_ALL = bool(int(os.environ.get("MOE_TRACE_ALL", "0")))
LAST = {}

_BUILD_CACHE = {}


def _enable_axon_profiling():
    """The image's antenv lacks axon_hooks, so boot() silently skipped NTFF
    hook registration. Recreate the module and register the ctypes hook so
    run_bass_kernel_spmd(trace=True) can profile. Also keep artifacts local."""
    import types

    if "antenv.axon_hooks" not in sys.modules:
        mod = types.ModuleType("antenv.axon_hooks")
        mod._hook = None

        def set_axon_ntff_profile_hook(h):
            mod._hook = h

        def get_axon_ntff_profile_hook():
            return mod._hook

        mod.set_axon_ntff_profile_hook = set_axon_ntff_profile_hook
        mod.get_axon_ntff_profile_hook = get_axon_ntff_profile_hook
        sys.modules["antenv.axon_hooks"] = mod
        import antenv

        antenv.axon_hooks = mod
    hooks = sys.modules["antenv.axon_hooks"]
    if hooks.get_axon_ntff_profile_hook() is None:
        from trn_agent_boot.trn_boot import _ntff_profile_via_ctypes

        hooks.set_axon_ntff_profile_hook(
            _ntff_profile_via_ctypes("/opt/axon/libaxon_pjrt.so")
        )
    import concourse.bass_utils as bu

    bu.upload_artifacts = lambda tmpdir: tmpdir


if TRACE:
    _enable_axon_profiling()


CC = 512


def _chunks_for(C):
    ch = [CC] * (C // CC)
    if C % CC:
        ch.append(C % CC)
    return ch


def _build(C, act_func=None):
    """One expert's FFN over C (padded) tokens; SPMD across 8 cores."""
    if act_func is None:
        act_func = mybir.ActivationFunctionType.Gelu
    nc = bacc.Bacc()
    dt = mybir.dt
    xTc = nc.dram_tensor("xTc", [P, ND, C], dt.bfloat16, kind="ExternalInput")
    w1c = nc.dram_tensor("w1c", [P, NF, ND, P], dt.bfloat16, kind="ExternalInput")
    w2c = nc.dram_tensor("w2c", [P, NF, D], dt.bfloat16, kind="ExternalInput")
    b1c = nc.dram_tensor("b1c", [P, NF], dt.float32, kind="ExternalInput")
    cwc = nc.dram_tensor("cwc", [P, C // P], dt.float32, kind="ExternalInput")
    y = nc.dram_tensor("y", [C, D], dt.float32, kind="ExternalOutput")

    chunks = _chunks_for(C)
    with tile.TileContext(nc) as tc:
        with (
            tc.tile_pool(name="weights", bufs=1) as wpool,
            tc.tile_pool(name="consts", bufs=1) as cpool,
            tc.tile_pool(name="xin", bufs=2) as xpool,
            tc.tile_pool(name="hmid", bufs=1) as hpool,
            tc.tile_pool(name="yout", bufs=3) as ypool,
            tc.tile_pool(name="psh", bufs=4, space="PSUM") as psh,
            tc.tile_pool(name="psy", bufs=4, space="PSUM") as psy,
        ):
            # w1 fb-major in 8 tiles of 4 F-blocks (1MB each): first matmul
            # group waits on ~1MB; DMA delivery stays ahead of the PE's
            # 6.9us-per-4-blocks consumption.
            w1_sb = [wpool.tile([P, 4, ND, P], dt.bfloat16, name=f"w1_{t}", tag=f"w1_{t}") for t in range(NF // 4)]
            w2_sb = [wpool.tile([P, 4, D], dt.bfloat16, name=f"w2_{g}", tag=f"w2_{g}") for g in range(NF // 4)]
            b1_sb = cpool.tile([P, NF], dt.float32)
            cw_sb = cpool.tile([P, C // P], dt.float32)

            # PE warmup (p-state ramp) on memset data, overlapping the DMAs.
            warm_l = cpool.tile([P, P], dt.bfloat16)
            nc.vector.memset(warm_l[:], 0.0)
            warm_ps = psy.tile([P, 512], dt.float32, tag="py")
            for i in range(10):
                nc.tensor.matmul(
                    warm_ps[:, :P], warm_l[:], warm_l[:],
                    start=(i == 0), stop=(i == 9),
                )

            # DMA issue order = consumption order: first w1 tile, then x
            # chunk 0 in two halves, then the rest of w1. Separate rings run
            # the transfers in parallel; ordering sets priority.
            xT0_sb = xpool.tile([P, ND, CC], dt.bfloat16, tag="xT")
            nc.sync.dma_start(out=w1_sb[0][:], in_=w1c[:, 0:4])
            for kh in range(2):
                nc.sync.dma_start(
                    out=xT0_sb[:, kh * 4 : (kh + 1) * 4, : chunks[0]],
                    in_=xTc[:, kh * 4 : (kh + 1) * 4, : chunks[0]],
                )
            for t in range(1, NF // 4):
                nc.sync.dma_start(out=w1_sb[t][:], in_=w1c[:, t * 4 : (t + 1) * 4])
            nc.sync.dma_start(out=b1_sb[:], in_=b1c[:])
            nc.sync.dma_start(out=cw_sb[:], in_=cwc[:])

            c0 = 0
            for ci, Cc in enumerate(chunks):
                ncb = Cc // P
                if ci == 0:
                    xT_sb = xT0_sb
                else:
                    xT_sb = xpool.tile([P, ND, CC], dt.bfloat16, tag="xT")
                    nc.sync.dma_start(
                        out=xT_sb[:, :, :Cc], in_=xTc[:, :, c0 : c0 + Cc]
                    )
                hT_sb = hpool.tile([P, NF, CC], dt.bfloat16, tag="hT")
                for fb in range(NF):
                    if ci == 0 and fb == 6:
                        # w2 streams in while chunk 0's m1 still has ~45us
                        # of work; ready well before m2 starts.
                        for g in range(NF // 4):
                            nc.sync.dma_start(
                                out=w2_sb[g][:],
                                in_=w2c[:, g * 4 : (g + 1) * 4, :],
                            )
                    ph = psh.tile([P, CC], dt.float32, tag="ph")
                    for kd in range(ND):
                        nc.tensor.matmul(
                            ph[:, :Cc],
                            w1_sb[fb // 4][:, fb % 4, kd, :],
                            xT_sb[:, kd, :Cc],
                            start=(kd == 0),
                            stop=(kd == ND - 1),
                        )
                    nc.scalar.activation(
                        hT_sb[:, fb, :Cc],
                        ph[:, :Cc],
                        act_func,
                        bias=b1_sb[:, fb : fb + 1],
                    )
                for cb in range(ncb):
                    y_sb = ypool.tile([P, D], dt.float32, tag="y")
                    for dc in range(2):
                        py = psy.tile([P, 512], dt.float32, tag="py")
                        for fb in range(NF):
                            nc.tensor.matmul(
                                py[:],
                                hT_sb[:, fb, cb * P : (cb + 1) * P],
                                w2_sb[fb // 4][:, fb % 4, dc * 512 : (dc + 1) * 512],
                                start=(fb == 0),
                                stop=(fb == NF - 1),
                            )
                        blk = c0 // P + cb
                        nc.vector.tensor_scalar_mul(
                            y_sb[:, dc * 512 : (dc + 1) * 512],
                            py[:],
                            cw_sb[:, blk : blk + 1],
                        )
                        nc.sync.dma_start(
                            out=y[
                                c0 + cb * P : c0 + (cb + 1) * P,
                                dc * 512 : (dc + 1) * 512,
                            ],
                            in_=y_sb[:, dc * 512 : (dc + 1) * 512],
                        )
                c0 += Cc
    nc.compile()
    return nc


def _route(xf, router_w, router_b):
    """Replicates reference routing in numpy f32."""
    logits = xf @ router_w + router_b
    logits = logits - logits.max(axis=1, keepdims=True)
    p = np.exp(logits)
    p /= p.sum(axis=1, keepdims=True)
    top_i = np.argsort(-p, axis=1, kind="stable")[:, :TOP_K]
    tp = np.take_along_axis(p, top_i, 1)
    tp = tp / tp.sum(axis=1, keepdims=True)
    return top_i, tp.astype(np.float32)


def kernel(x, w1, b1, w2, b2, router_w, router_b):
    x = np.asarray(x, np.float32)
    B, S, _ = x.shape
    T = B * S
    xf = x.reshape(T, D)
    w1f = np.asarray(w1, np.float32)
    w2f = np.asarray(w2, np.float32)
    b1f = np.asarray(b1, np.float32)
    b2f = np.asarray(b2, np.float32)

    top_i, tp = _route(xf, np.asarray(router_w, np.float32), np.asarray(router_b, np.float32))

    idxs, cws = [], []
    for e in range(E):
        sel = top_i == e
        rows = np.nonzero(sel.any(axis=1))[0]
        w = (tp * sel).sum(axis=1)[rows]
        idxs.append(rows)
        cws.append(w.astype(np.float32))

    maxn = max(len(r) for r in idxs)
    C = max(CC, ((maxn + 127) // 128) * 128)

    if C not in _BUILD_CACHE:
        _BUILD_CACHE[C] = _build(C)
    nc = _BUILD_CACHE[C]

    w1b = w1f.astype(BF16)
    w2b = w2f.astype(BF16)
    in_maps = []
    for e in range(E):
        n = len(idxs[e])
        xT = np.zeros((P, ND, C), BF16)
        if n:
            g = xf[idxs[e]].astype(BF16).T  # [D, n]
            xT[:, :, :n] = g.reshape(ND, P, n).transpose(1, 0, 2)
        cwf = np.zeros(C, np.float32)
        cwf[:n] = cws[e]
        in_maps.append(
            {
                "xTc": xT,
                # [P, NF, ND, P]: w1c[p, fb, kd, c] = w1[kd*P + p, fb*P + c]
                "w1c": np.ascontiguousarray(w1b[e].reshape(ND, P, NF, P).transpose(1, 2, 0, 3)),
                "w2c": np.ascontiguousarray(w2b[e].reshape(NF, P, D).transpose(1, 0, 2)),
                "b1c": np.ascontiguousarray(b1f[e].reshape(NF, P).T),
                "cwc": np.ascontiguousarray(cwf.reshape(C // P, P).T),
            }
        )

    res = run_bass_kernel_spmd(
        nc,
        in_maps,
        list(range(E)),
        trace=TRACE,
        trace_cores=list(range(E)) if TRACE_ALL else None,
    )
    LAST["exec_time_ns"] = res.exec_time_ns
    LAST["res"] = res
    LAST["C"] = C

    outf = np.zeros((T, D), np.float32)
    for e in range(E):
        n = len(idxs[e])
        if n:
            ye = np.asarray(res.results[e]["y"], np.float32)
            outf[idxs[e]] += ye[:n]
    # b2 enters as sum_e cw[e,t] * b2[e]
    cw_dense = np.zeros((T, E), np.float32)
    np.put_along_axis(cw_dense, top_i, tp, axis=1)
    outf += cw_dense @ b2f
    return outf.reshape(B, S, D)
